# revision 1
# baseline (speedup 1.0000x reference)
"""GCN (3-layer, PyG GCNConv semantics) on 8 Trainium2 NeuronCores.

Sharding: nodes are partitioned across the 8 cores by destination id
(graph-parallel). Each core aggregates messages for its own node shard; the
per-layer node features ("tables") are replicated via chunked AllGathers so
every core can gather arbitrary source rows with dma_gather (int16 indices,
so the table is split into 4 row-chunks < 2^15 rows; each chunk is exactly
one sub-AllGather region, letting the collectives overlap the producing
compute).

Math (A_hat = D^-1/2 (A+I) D^-1/2): per layer
    out = dinv * (agg of z) [@ W] + b,  z = dinv * (h @ W)
(W folded before aggregation for layers 1/2, after for layer 3), where
    agg_n = z_n + sum_{e: dst=n} z_src.

Device pipeline per core: L0 computes z1^T per 256-node group
(feature-major); each aggregation layer gathers 128-edge tiles from the
table, builds a selection matrix S[e, j] = (dstoff[e] == j) on DVE and
accumulates psum[64, 256] with f32r matmuls (lhsT = gathered rows,
rhs = S); group epilogues run feature-major, and PE transposes convert
back to node-major shard rows for the next table.
"""
import sys
sys.path.insert(0, '/opt/trn_rl_repo')

from contextlib import ExitStack

import numpy as np

from concourse import bass, bacc, tile, mybir, library_config
from concourse.bass_utils import run_bass_kernel_spmd
from concourse.masks import make_identity

# ---- problem constants (hardcoded) ----
N_NODES = 100000
IN_DIM, HID_DIM, OUT_DIM = 128, 64, 7
N_CORES = 8
NS_RAW = N_NODES // N_CORES          # 12500 real nodes per core
P = 128
G = 256                              # nodes per aggregation group
NGRP = 49                            # 12544 / 256
NS = NGRP * G                        # 12544 padded shard size
V = NS * N_CORES                     # 100352 table rows
SUB_GRPS = [13, 12, 12, 12]          # groups per sub-AllGather / idx chunk
N_CHUNK = len(SUB_GRPS)
WAVE_SPLITS = {13: [5, 4, 4], 12: [4, 4, 4]}
NI_MAX = 1024                        # dma_gather idxs per instruction cap

f32 = mybir.dt.float32
f32r = mybir.dt.float32r
i16 = mybir.dt.int16

SUB_G0 = np.cumsum([0] + SUB_GRPS)           # group start per sub
SUB_ROWS = [g * G for g in SUB_GRPS]         # shard rows per sub
SUB_R0 = np.cumsum([0] + SUB_ROWS)           # shard row start per sub
CHUNK_ROWS = [r * N_CORES for r in SUB_ROWS]  # table rows per chunk
CHUNK_BASE = np.cumsum([0] + CHUNK_ROWS)
SUB_OF_GROUP = np.concatenate(
    [np.full(n, s, dtype=np.int64) for s, n in enumerate(SUB_GRPS)])

# waves: list of (sub, [groups])
WAVES = []
for s in range(N_CHUNK):
    g0 = SUB_G0[s]
    for w in WAVE_SPLITS[SUB_GRPS[s]]:
        WAVES.append((s, list(range(g0, g0 + w))))
        g0 += w
WGRP_MAX = max(len(wg) for _, wg in WAVES)


def _host_prep(edge_index):
    """Partition/permute/pad the graph into a static structure shared by all
    cores (cores differ only in input data, not program shape)."""
    src = np.asarray(edge_index[0], dtype=np.int64)
    dst = np.asarray(edge_index[1], dtype=np.int64)
    deg = np.bincount(dst, minlength=N_NODES).astype(np.float64) + 1.0
    dinv = (1.0 / np.sqrt(deg)).astype(np.float32)

    core_of = dst // NS_RAW
    perms = []
    for c in range(N_CORES):
        sel = core_of == c
        dl = (dst[sel] - c * NS_RAW).astype(np.int64)
        cnt = np.bincount(dl, minlength=NS)
        order = np.argsort(-cnt, kind='stable')
        gload = np.zeros(NGRP, dtype=np.int64)
        gfill = np.zeros(NGRP, dtype=np.int64)
        perm = np.empty(NS, dtype=np.int64)
        for node in order:
            cand = np.where(gfill < G)[0]
            gsel = cand[np.argmin(gload[cand])]
            perm[node] = gsel * G + gfill[gsel]
            gfill[gsel] += 1
            gload[gsel] += cnt[node]
        perms.append(perm)

    def trow_chunkidx(nodes):
        """global node id -> (chunk, in-chunk row)"""
        c = nodes // NS_RAW
        loc = np.empty(len(nodes), dtype=np.int64)
        for cc in range(N_CORES):
            m = c == cc
            loc[m] = perms[cc][nodes[m] % NS_RAW]
        g = loc // G
        s = SUB_OF_GROUP[g]
        inrow = c * np.array(SUB_ROWS)[s] + (loc - SUB_R0[s])
        return s, inrow

    src_ch, src_row = trow_chunkidx(src)

    # per-core (group, chunk) runs
    run_lens = np.zeros((N_CORES, NGRP, N_CHUNK), dtype=np.int64)
    edge_lists = []
    for c in range(N_CORES):
        sel = core_of == c
        rows_c, ch_c = src_row[sel], src_ch[sel]
        d_new = perms[c][(dst[sel] - c * NS_RAW)]
        grp, off = d_new // G, d_new % G
        runs = {}
        for g in range(NGRP):
            gm = grp == g
            for k in range(N_CHUNK):
                m = gm & (ch_c == k)
                runs[(g, k)] = (rows_c[m], off[m])
                run_lens[c, g, k] = m.sum()
        edge_lists.append(runs)

    # static tiles per (group, chunk): max over cores
    t_arr = np.ceil(run_lens.max(axis=0) / P).astype(np.int64)  # [NGRP, N_CHUNK]

    # flat tile order + instruction plan (identical for all cores)
    instr_plan = []   # (wave_idx, chunk, tile_off_in_wavechunk, n_idx, idx_col0)
    tiles_meta = []   # flat: (wave_idx, chunk, group, j)
    wave_tiles = []   # tiles per wave
    idx_cols = 0
    for wi, (s, wg) in enumerate(WAVES):
        wt = 0
        for k in range(N_CHUNK):
            slots = int(t_arr[wg, k].sum()) * P
            done = 0
            while done < slots:
                ni = min(NI_MAX, slots - done)
                instr_plan.append((wi, k, done // P, ni, idx_cols))
                idx_cols += ni // 16
                done += ni
            for g in wg:
                for j in range(int(t_arr[g, k])):
                    tiles_meta.append((wi, k, g, j))
                wt += int(t_arr[g, k])
        wave_tiles.append(wt)
    tt = len(tiles_meta)

    # flat slot base of each (wave, chunk) region
    wc_tilebase = {}
    ti = 0
    for wi, (s, wg) in enumerate(WAVES):
        for k in range(N_CHUNK):
            wc_tilebase[(wi, k)] = ti
            ti += int(t_arr[wg, k].sum())

    idx_arrs, doff_arrs = [], []
    for c in range(N_CORES):
        runs = edge_lists[c]
        flat_idx = np.zeros(tt * P, dtype=np.int16)
        flat_off = np.full(tt * P, -1.0, dtype=np.float32)
        pos = 0
        for (wi, k, g, j) in tiles_meta:
            rows, offs = runs[(g, k)]
            a, b = j * P, min((j + 1) * P, len(rows))
            n = max(0, b - a)
            if n > 0:
                flat_idx[pos:pos + n] = rows[a:b].astype(np.int16)
                flat_off[pos:pos + n] = offs[a:b].astype(np.float32)
            pos += P
        idx_wrapped = np.zeros((P, idx_cols), dtype=np.int16)
        for (wi, k, toff, ni, col0) in instr_plan:
            s0 = (wc_tilebase[(wi, k)] + toff) * P
            blk = flat_idx[s0:s0 + ni].reshape(ni // 16, 16).T
            idx_wrapped[:, col0:col0 + ni // 16] = np.tile(blk, (8, 1))
        idx_arrs.append(idx_wrapped)
        doff_arrs.append(flat_off.reshape(tt, P).T.copy())

    return dict(
        dinv=dinv, perms=perms, t_arr=t_arr, instr_plan=instr_plan,
        tiles_meta=tiles_meta, tt=tt, wave_tiles=wave_tiles,
        wc_tilebase=wc_tilebase, idx_arrs=idx_arrs, doff_arrs=doff_arrs,
        idx_cols=idx_cols,
    )


def _build_program(S, repeat=1, ag_mode='collective'):
    t_arr, instr_plan, tt, idx_cols = (
        S['t_arr'], S['instr_plan'], S['tt'], S['idx_cols'])
    wave_tiles, wc_tilebase = S['wave_tiles'], S['wc_tilebase']
    mw_tiles_max = max(wave_tiles)

    nc = bacc.Bacc("TRN2", target_bir_lowering=False, debug=False,
                   num_devices=N_CORES)

    xT_d = nc.dram_tensor("xT", [P, NS], f32r, kind="ExternalInput")
    dinvrep_d = nc.dram_tensor("dinvrep", [HID_DIM, NS], f32, kind="ExternalInput")
    idx_d = nc.dram_tensor("idx", [P, idx_cols], i16, kind="ExternalInput")
    doff_d = nc.dram_tensor("doff", [P, tt], f32, kind="ExternalInput")
    W1_d = nc.dram_tensor("W1", [IN_DIM, HID_DIM], f32r, kind="ExternalInput")
    W2_d = nc.dram_tensor("W2", [HID_DIM, HID_DIM], f32r, kind="ExternalInput")
    W3_d = nc.dram_tensor("W3", [HID_DIM, 8], f32r, kind="ExternalInput")
    b1_d = nc.dram_tensor("b1", [HID_DIM, 1], f32, kind="ExternalInput")
    b2_d = nc.dram_tensor("b2", [HID_DIM, 1], f32, kind="ExternalInput")
    b3_d = nc.dram_tensor("b3", [8, 1], f32, kind="ExternalInput")
    out_d = nc.dram_tensor("out_shard", [NS, 8], f32, kind="ExternalOutput")

    nc.gpsimd.load_library(library_config.mlp)

    with tile.TileContext(nc) as tc:
        stack = ExitStack()
        zsh = [tc.tile([NS, HID_DIM], f32r, space="DRAM", name=f"zsh{i}")[0]
               for i in range(3)]
        dramp = stack.enter_context(
            tc.tile_pool(name="dramp", bufs=1, space="DRAM"))
        def alloc_tables(rep):
            return [[dramp.tile([CHUNK_ROWS[k], HID_DIM], f32r,
                                addr_space="Shared",
                                name=f"table{rep}_{i}_{k}",
                                tag=f"table{rep}_{i}_{k}")
                     for k in range(N_CHUNK)] for i in range(3)]
        const = stack.enter_context(tc.tile_pool(name="const", bufs=1))

        R_i = const.tile([P, G], mybir.dt.int32)
        nc.gpsimd.iota(R_i[:], pattern=[[1, G]], base=0, channel_multiplier=0)
        R_f = const.tile([P, G], f32)
        nc.vector.tensor_copy(out=R_f[:], in_=R_i[:])
        ident = const.tile([P, P], f32)
        make_identity(nc, ident[:])
        ident_r = const.tile([P, P], f32r)
        nc.vector.tensor_copy(out=ident_r[:], in_=ident[:])

        W1_t = const.tile([IN_DIM, HID_DIM], f32r)
        nc.sync.dma_start(out=W1_t[:], in_=W1_d[:])
        W2_t = const.tile([HID_DIM, HID_DIM], f32r)
        nc.sync.dma_start(out=W2_t[:], in_=W2_d[:])
        W3_t = const.tile([HID_DIM, 8], f32r)
        nc.sync.dma_start(out=W3_t[:], in_=W3_d[:])
        b1_t = const.tile([HID_DIM, 1], f32)
        nc.sync.dma_start(out=b1_t[:], in_=b1_d[:])
        b2_t = const.tile([HID_DIM, 1], f32)
        nc.sync.dma_start(out=b2_t[:], in_=b2_d[:])
        b3_t = const.tile([8, 1], f32)
        nc.sync.dma_start(out=b3_t[:], in_=b3_d[:])
        idx_t = const.tile([P, idx_cols], i16)
        nc.sync.dma_start(out=idx_t[:], in_=idx_d[:])
        doff_t = const.tile([P, tt], f32)
        nc.sync.dma_start(out=doff_t[:], in_=doff_d[:])

        zTd = [tc.tile([HID_DIM, NS], f32r, space="DRAM", name=f"zTd{i}")[0]
               for i in range(2)]

        sbuf = stack.enter_context(tc.tile_pool(name="sbuf", bufs=3))
        spool = stack.enter_context(tc.tile_pool(name="spool", bufs=6))
        wavep = stack.enter_context(tc.tile_pool(name="wavep", bufs=2))
        znodep = stack.enter_context(tc.tile_pool(name="znodep", bufs=2))
        psum_agg = stack.enter_context(
            tc.tile_pool(name="psum_agg", bufs=3, space="PSUM"))
        psum_mm2 = stack.enter_context(
            tc.tile_pool(name="psum_mm2", bufs=2, space="PSUM"))
        psum_tr = stack.enter_context(
            tc.tile_pool(name="psum_tr", bufs=2, space="PSUM"))

        def load_dvw(wg):
            w0, wn = wg[0] * G, len(wg) * G
            dvw = wavep.tile([HID_DIM, wn], f32, tag="dvw",
                             padded_shape=[HID_DIM, WGRP_MAX * G])
            nc.sync.dma_start(out=dvw[:], in_=dinvrep_d[:, w0:w0 + wn])
            return dvw

        def store_wave_fm(zcw, wg, fdim, node_dram, zT_target):
            """Batch-transpose the feature-major wave tile [fdim, wn] into
            node-major [wn, fdim] rows of node_dram; also stash feature-major
            into zT_target if given."""
            w0, wn = wg[0] * G, len(wg) * G
            nch = wn // P
            for blk0 in range(0, nch, 8):
                nb = min(8, nch - blk0)
                ptr = psum_tr.tile([P, nb * fdim], f32r, tag="ptr",
                                   padded_shape=[P, 8 * HID_DIM])
                for i in range(nb):
                    nc.tensor.transpose(
                        out=ptr[:, i * fdim:(i + 1) * fdim],
                        in_=zcw[:fdim, (blk0 + i) * P:(blk0 + i + 1) * P],
                        identity=ident_r[:fdim, :fdim])
                zn = znodep.tile([P, nb * fdim], f32r, tag="zn",
                                 padded_shape=[P, 8 * HID_DIM])
                nc.vector.tensor_copy(out=zn[:], in_=ptr[:])
                dst = node_dram[w0 + blk0 * P: w0 + (blk0 + nb) * P, :]
                src_ap = zn[:] if node_dram is not out_d else zn[:].bitcast(f32)
                nc.scalar.dma_start(
                    out=dst.rearrange("(c p) f -> p c f", p=P),
                    in_=src_ap.rearrange("p (c f) -> p c f", f=fdim))
            if zT_target is not None:
                nc.scalar.dma_start(out=zT_target[:, w0:w0 + wn], in_=zcw[:])

        def sub_allgather(zsh_t, table_t, s):
            r0, rn = SUB_R0[s], SUB_ROWS[s]
            if ag_mode == 'local':
                nc.scalar.dma_start(out=table_t[s][0:rn, :],
                                    in_=zsh_t[r0:r0 + rn, :])
                return
            nc.gpsimd.collective_compute(
                "AllGather", mybir.AluOpType.bypass,
                replica_groups=[list(range(N_CORES))],
                ins=[zsh_t[r0:r0 + rn, :]],
                outs=[table_t[s][:]])

        for _rep in range(repeat):
            tables = alloc_tables(_rep)
            # ---------- L0: z1 = dinv .* (x @ W1), feature-major ----------
            for wi, (s, wg) in enumerate(WAVES):
                w0, wn = wg[0] * G, len(wg) * G
                xw = wavep.tile([P, wn], f32r, tag="xw",
                                padded_shape=[P, WGRP_MAX * G])
                nc.sync.dma_start(out=xw[:], in_=xT_d[:, w0:w0 + wn])
                dvw = load_dvw(wg)
                zcw = wavep.tile([HID_DIM, wn], f32r, tag="zcw",
                                 padded_shape=[HID_DIM, WGRP_MAX * G])
                for g in wg:
                    c0 = (g - wg[0]) * G
                    ps = psum_agg.tile([HID_DIM, G], f32, tag="ps")
                    nc.tensor.matmul(out=ps[:], lhsT=W1_t[:],
                                     rhs=xw[:, c0:c0 + G],
                                     start=True, stop=True)
                    nc.vector.tensor_tensor(
                        out=zcw[:, c0:c0 + G], in0=ps[:],
                        in1=dvw[:, c0:c0 + G], op=mybir.AluOpType.mult)
                store_wave_fm(zcw, wg, HID_DIM, zsh[0], zTd[0])
                if wi + 1 == len(WAVES) or WAVES[wi + 1][0] != s:
                    sub_allgather(zsh[0], tables[0], s)

            # ---------- aggregation layers ----------
            def agg_layer(layer, table, zT_in, zT_out_d, W_next, bias_t,
                          final=False):
                for wi, (s, wg) in enumerate(WAVES):
                    w0, wn = wg[0] * G, len(wg) * G
                    wtiles = wave_tiles[wi]
                    mw = wavep.tile([P, wtiles, HID_DIM], f32r, tag="mw",
                                    padded_shape=[P, mw_tiles_max, HID_DIM])
                    wave_t0 = wc_tilebase[(wi, 0)]
                    for (wi2, k, toff, ni, col0) in instr_plan:
                        if wi2 != wi:
                            continue
                        ck = wc_tilebase[(wi, k)] - wave_t0
                        nc.gpsimd.dma_gather(
                            out_ap=mw[:, ck + toff: ck + toff + ni // P, :],
                            in_ap=table[k][:],
                            idxs_ap=idx_t[:, col0: col0 + ni // 16],
                            num_idxs=ni, num_idxs_reg=ni, elem_size=HID_DIM,
                            single_packet=True,
                        )
                    zsw = wavep.tile([HID_DIM, wn], f32r, tag="zsw",
                                     padded_shape=[HID_DIM, WGRP_MAX * G])
                    nc.sync.dma_start(out=zsw[:], in_=zT_in[:, w0:w0 + wn])
                    dvw = load_dvw(wg)
                    if final:
                        zcw = wavep.tile([8, wn], f32r, tag="ocw",
                                         padded_shape=[8, WGRP_MAX * G])
                    else:
                        zcw = wavep.tile([HID_DIM, wn], f32r, tag="zcw",
                                         padded_shape=[HID_DIM, WGRP_MAX * G])
                    for gi, g in enumerate(wg):
                        ps = psum_agg.tile([HID_DIM, G], f32, tag="ps")
                        n_mm = int(t_arr[g].sum())
                        mm_i = 0
                        for k in range(N_CHUNK):
                            ck = wc_tilebase[(wi, k)] - wave_t0
                            jbase = int(t_arr[wg[0]:g, k].sum())
                            for j in range(int(t_arr[g, k])):
                                wt = ck + jbase + j
                                ft = wave_t0 + wt if k == 0 else (
                                    wc_tilebase[(wi, k)] + jbase + j)
                                St = spool.tile([P, G], f32r, tag="St")
                                nc.vector.tensor_scalar(
                                    out=St[:], in0=R_f[:],
                                    scalar1=doff_t[:, ft:ft + 1], scalar2=None,
                                    op0=mybir.AluOpType.is_equal)
                                nc.tensor.matmul(
                                    out=ps[:], lhsT=mw[:, wt, :], rhs=St[:],
                                    start=(mm_i == 0), stop=(mm_i == n_mm - 1))
                                mm_i += 1
                        # ---- epilogue for group g ----
                        c0 = (g - wg[0]) * G
                        c1 = c0 + G
                        e1 = sbuf.tile([HID_DIM, G], f32, tag="e1")
                        nc.vector.tensor_tensor(out=e1[:], in0=ps[:],
                                                in1=zsw[:, c0:c1],
                                                op=mybir.AluOpType.add)
                        if final:
                            e2 = sbuf.tile([HID_DIM, G], f32r, tag="e2")
                            nc.vector.tensor_tensor(out=e2[:], in0=e1[:],
                                                    in1=dvw[:, c0:c1],
                                                    op=mybir.AluOpType.mult)
                            po = psum_mm2.tile([8, G], f32, tag="po")
                            nc.tensor.matmul(out=po[:], lhsT=W3_t[:],
                                             rhs=e2[:], start=True, stop=True)
                            nc.vector.tensor_scalar(
                                out=zcw[:, c0:c1], in0=po[:],
                                scalar1=b3_t[:, :1],
                                scalar2=None, op0=mybir.AluOpType.add)
                        else:
                            e2 = sbuf.tile([HID_DIM, G], f32, tag="e2")
                            nc.vector.tensor_tensor(out=e2[:], in0=e1[:],
                                                    in1=dvw[:, c0:c1],
                                                    op=mybir.AluOpType.mult)
                            hT = sbuf.tile([HID_DIM, G], f32r, tag="hT")
                            nc.vector.tensor_scalar(
                                out=hT[:], in0=e2[:], scalar1=bias_t[:, :1],
                                scalar2=0.0, op0=mybir.AluOpType.add,
                                op1=mybir.AluOpType.max)
                            if W_next is not None:
                                po = psum_mm2.tile([HID_DIM, G], f32, tag="po")
                                nc.tensor.matmul(out=po[:], lhsT=W_next[:],
                                                 rhs=hT[:], start=True,
                                                 stop=True)
                                nc.vector.tensor_tensor(
                                    out=zcw[:, c0:c1], in0=po[:],
                                    in1=dvw[:, c0:c1],
                                    op=mybir.AluOpType.mult)
                            else:
                                nc.vector.tensor_tensor(
                                    out=zcw[:, c0:c1], in0=hT[:],
                                    in1=dvw[:, c0:c1],
                                    op=mybir.AluOpType.mult)
                    if final:
                        store_wave_fm(zcw, wg, 8, out_d, None)
                    else:
                        store_wave_fm(zcw, wg, HID_DIM, zsh[layer],
                                      zT_out_d)
                        if wi + 1 == len(WAVES) or WAVES[wi + 1][0] != s:
                            sub_allgather(zsh[layer], tables[layer], s)

            agg_layer(1, tables[0], zTd[0], zTd[1], W2_t, b1_t)
            agg_layer(2, tables[1], zTd[1], zTd[0], None, b2_t)
            agg_layer(3, tables[2], zTd[0], None, None, b3_t, final=True)
        stack.close()

    nc.finalize()
    return nc


def _make_in_maps(S, x, W1, b1, W2, b2, W3, b3):
    dinv = S['dinv']
    W3p = np.zeros((HID_DIM, 8), np.float32)
    W3p[:, :OUT_DIM] = W3
    b3p = np.zeros((8, 1), np.float32)
    b3p[:OUT_DIM, 0] = b3
    in_maps = []
    for c in range(N_CORES):
        perm = S['perms'][c]
        xs = np.zeros((NS, IN_DIM), np.float32)
        dv = np.ones(NS, np.float32)
        xs[perm[:NS_RAW]] = x[c * NS_RAW:(c + 1) * NS_RAW]
        dv[perm[:NS_RAW]] = dinv[c * NS_RAW:(c + 1) * NS_RAW]
        in_maps.append({
            "xT": np.ascontiguousarray(xs.T),
            "dinvrep": np.ascontiguousarray(
                np.broadcast_to(dv[None, :], (HID_DIM, NS))),
            "idx": S['idx_arrs'][c],
            "doff": S['doff_arrs'][c],
            "W1": W1, "W2": W2, "W3": W3p,
            "b1": b1.reshape(-1, 1), "b2": b2.reshape(-1, 1), "b3": b3p,
        })
    return in_maps


_LAST = {}


def kernel(x, edge_index, W1, b1, W2, b2, W3, b3):
    x = np.asarray(x, dtype=np.float32)
    W1 = np.asarray(W1, dtype=np.float32)
    W2 = np.asarray(W2, dtype=np.float32)
    W3 = np.asarray(W3, dtype=np.float32)
    b1 = np.asarray(b1, dtype=np.float32)
    b2 = np.asarray(b2, dtype=np.float32)
    b3 = np.asarray(b3, dtype=np.float32)

    S = _host_prep(edge_index)
    nc = _build_program(S)
    in_maps = _make_in_maps(S, x, W1, b1, W2, b2, W3, b3)

    res = run_bass_kernel_spmd(nc, in_maps, core_ids=list(range(N_CORES)))

    _LAST['S'] = S
    _LAST['in_maps'] = in_maps

    out = np.empty((N_NODES, OUT_DIM), np.float32)
    for c in range(N_CORES):
        shard = res.results[c]["out_shard"]       # [NS, 8]
        perm = S['perms'][c]
        out[c * NS_RAW:(c + 1) * NS_RAW] = shard[perm[:NS_RAW], :OUT_DIM]
    return out


def measure_exec_ns(repeats=(1, 5), iters=6, ag_mode='collective'):
    """Estimate HW exec time by building R-times-repeated variants of the
    full pipeline and differencing pipelined wall-clock."""
    import time
    import jax
    from jax.sharding import Mesh, PartitionSpec, NamedSharding
    from jax.experimental.shard_map import shard_map
    from concourse import bass2jax
    from concourse.bass2jax import _bass_exec_p, install_neuronx_cc_hook

    S, in_maps = _LAST['S'], _LAST['in_maps']
    install_neuronx_cc_hook()
    per_call = {}
    for R in repeats:
        nc = _build_program(S, repeat=R, ag_mode=ag_mode)
        partition_name = (nc.partition_id_tensor.name
                          if nc.partition_id_tensor else None)
        in_names, out_names, out_avals, zero_outs = [], [], [], []
        for alloc in nc.m.functions[0].allocations:
            if not isinstance(alloc, mybir.MemoryLocationSet):
                continue
            name = alloc.memorylocations[0].name
            if alloc.kind == "ExternalInput":
                if name != partition_name:
                    in_names.append(name)
            elif alloc.kind == "ExternalOutput":
                out_names.append(name)
                shape = tuple(alloc.tensor_shape)
                dtype = mybir.dt.np(alloc.dtype)
                out_avals.append(jax.core.ShapedArray(shape, dtype))
                zero_outs.append(np.zeros(shape, dtype))
        all_in = list(in_names) + list(out_names)
        if partition_name:
            all_in.append(partition_name)

        def _body(*args, _nc=nc, _avals=tuple(out_avals), _in=tuple(all_in),
                  _out=tuple(out_names)):
            operands = list(args)
            operands.append(bass2jax.partition_id_tensor())
            return tuple(_bass_exec_p.bind(
                *operands, out_avals=_avals, in_names=_in, out_names=_out,
                lowering_input_output_aliases=(), sim_require_finite=True,
                sim_require_nnan=True, nc=_nc))

        devices = jax.devices()[:N_CORES]
        mesh = Mesh(np.asarray(devices), ("core",))
        nsp = len(in_names) + len(zero_outs)
        sharded = jax.jit(shard_map(
            _body, mesh=mesh, in_specs=(PartitionSpec("core"),) * nsp,
            out_specs=(PartitionSpec("core"),) * len(out_names),
            check_rep=False), keep_unused=True)
        args = [np.concatenate([np.asarray(in_maps[c][n]) for c in
                                range(N_CORES)], axis=0) for n in in_names]
        args += [np.zeros((N_CORES * z.shape[0], *z.shape[1:]), z.dtype)
                 for z in zero_outs]
        sh = NamedSharding(mesh, PartitionSpec("core"))
        args = [jax.device_put(a, sh) for a in args]
        outs = sharded(*args)
        jax.block_until_ready(outs)
        best = None
        for _ in range(iters):
            t0 = time.perf_counter()
            got = [sharded(*args) for _ in range(4)]
            jax.block_until_ready(got)
            dt = (time.perf_counter() - t0) / 4
            best = dt if best is None else min(best, dt)
        per_call[R] = best
    r0, r1 = repeats
    est = (per_call[r1] - per_call[r0]) / (r1 - r0)
    return max(1, int(est * 1e9))



# revision 2
# speedup vs baseline: 1.9019x; 1.9019x over previous
"""GCN (3-layer, PyG GCNConv semantics) on 8 Trainium2 NeuronCores — v2.

Sharding: nodes partitioned across 8 cores by destination id. Aggregation per
128-edge tile: dma_gather rows from a fp16 DRAM table (256B row stride),
matmul with a one-hot selection matrix S[e, dstoff] into PSUM.

v2 changes vs baseline:
  - Layer 1 gathers directly from a host-staged dinv-prescaled x table
    (fp16 [V,128], replicated input => no layer-1 AllGather at all);
    W1 applied after aggregation (it factors out of the sum).
  - fp16 everywhere on the table path: halves AllGather wire bytes, halves
    DVE cost of the S builds. PSUM accumulation stays f32.
  - Tables are AllGathered packed (fp16 [rows,64] / [rows,8]) and locally
    re-strided to 256B-padded rows via a DVE strided copy (contiguous DMAs).
  - Layer-3 table is 8-wide (W3 folded before the AllGather): tiny collective.
  - Layers 2/3 consume the table in two chunk-pair passes so each sub
    AllGather overlaps compute of the previous pass/layer.
"""
import sys
sys.path.insert(0, '/opt/trn_rl_repo')

from contextlib import ExitStack

import numpy as np

from concourse import bass, bacc, tile, mybir, library_config
from concourse.bass_utils import run_bass_kernel_spmd
from concourse.masks import make_identity

# ---- problem constants (hardcoded) ----
N_NODES = 100000
IN_DIM, HID_DIM, OUT_DIM = 128, 64, 7
N_CORES = 8
NS_RAW = N_NODES // N_CORES          # 12500 real nodes per core
P = 128
G = 256                              # nodes per aggregation group
NGRP = 49                            # 12544 / 256
NS = NGRP * G                        # 12544 padded shard size
V = NS * N_CORES                     # 100352 table rows
SUB_GRPS = [13, 12, 12, 12]          # groups per sub-AllGather / idx chunk
N_CHUNK = len(SUB_GRPS)
WAVE_SPLITS = {13: [5, 4, 4], 12: [4, 4, 4]}
NI_MAX = 1024                        # dma_gather idxs per instruction cap
PASSES = [(0, 1), (2, 3)]            # chunk-pair passes for layers 2/3

f32 = mybir.dt.float32
f32r = mybir.dt.float32r
f16 = mybir.dt.float16
i16 = mybir.dt.int16

SUB_G0 = np.cumsum([0] + SUB_GRPS)           # group start per sub
SUB_ROWS = [g * G for g in SUB_GRPS]         # shard rows per sub
SUB_R0 = np.cumsum([0] + SUB_ROWS)           # shard row start per sub
CHUNK_ROWS = [r * N_CORES for r in SUB_ROWS]  # table rows per chunk
CHUNK_BASE = np.cumsum([0] + CHUNK_ROWS)
SUB_OF_GROUP = np.concatenate(
    [np.full(n, s, dtype=np.int64) for s, n in enumerate(SUB_GRPS)])

# waves: list of (sub, [groups])
WAVES = []
for s in range(N_CHUNK):
    g0 = SUB_G0[s]
    for w in WAVE_SPLITS[SUB_GRPS[s]]:
        WAVES.append((s, list(range(g0, g0 + w))))
        g0 += w
N_WAVES = len(WAVES)


def _host_prep(edge_index):
    """Partition/permute/pad the graph into a static structure shared by all
    cores (cores differ only in input data, not program shape)."""
    src = np.asarray(edge_index[0], dtype=np.int64)
    dst = np.asarray(edge_index[1], dtype=np.int64)
    deg = np.bincount(dst, minlength=N_NODES).astype(np.float64) + 1.0
    dinv = (1.0 / np.sqrt(deg)).astype(np.float32)

    core_of = dst // NS_RAW
    perms = []
    for c in range(N_CORES):
        sel = core_of == c
        dl = (dst[sel] - c * NS_RAW).astype(np.int64)
        cnt = np.bincount(dl, minlength=NS)
        order = np.argsort(-cnt, kind='stable')
        gload = np.zeros(NGRP, dtype=np.int64)
        gfill = np.zeros(NGRP, dtype=np.int64)
        perm = np.empty(NS, dtype=np.int64)
        for node in order:
            cand = np.where(gfill < G)[0]
            gsel = cand[np.argmin(gload[cand])]
            perm[node] = gsel * G + gfill[gsel]
            gfill[gsel] += 1
            gload[gsel] += cnt[node]
        perms.append(perm)

    def trow_chunkidx(nodes):
        """global node id -> (chunk, in-chunk row)"""
        c = nodes // NS_RAW
        loc = np.empty(len(nodes), dtype=np.int64)
        for cc in range(N_CORES):
            m = c == cc
            loc[m] = perms[cc][nodes[m] % NS_RAW]
        g = loc // G
        s = SUB_OF_GROUP[g]
        inrow = c * np.array(SUB_ROWS)[s] + (loc - SUB_R0[s])
        return s, inrow

    src_ch, src_row = trow_chunkidx(src)

    # per-core (group, chunk) runs
    run_lens = np.zeros((N_CORES, NGRP, N_CHUNK), dtype=np.int64)
    edge_lists = []
    for c in range(N_CORES):
        sel = core_of == c
        rows_c, ch_c = src_row[sel], src_ch[sel]
        d_new = perms[c][(dst[sel] - c * NS_RAW)]
        grp, off = d_new // G, d_new % G
        runs = {}
        for g in range(NGRP):
            gm = grp == g
            for k in range(N_CHUNK):
                m = gm & (ch_c == k)
                runs[(g, k)] = (rows_c[m], off[m])
                run_lens[c, g, k] = m.sum()
        edge_lists.append(runs)

    # static tiles per (group, chunk): max over cores
    t_arr = np.ceil(run_lens.max(axis=0) / P).astype(np.int64)  # [NGRP, N_CHUNK]

    # ---- unified flat tile order: (wave, chunk) blocks, (g, j) within ----
    # Both the wave-major (L1) and chunk-pair-major (L2/L3) consumption
    # orders reference the same (wave, chunk) blocks, so one idx/doff layout
    # serves all layers.
    tiles_meta = []          # flat: (wi, k, g, j)
    wk_tilebase = {}         # (wi, k) -> flat tile base
    wk_tiles = {}            # (wi, k) -> tile count
    ti = 0
    for wi, (s, wg) in enumerate(WAVES):
        for k in range(N_CHUNK):
            wk_tilebase[(wi, k)] = ti
            n = int(t_arr[wg, k].sum())
            wk_tiles[(wi, k)] = n
            for g in wg:
                for j in range(int(t_arr[g, k])):
                    tiles_meta.append((wi, k, g, j))
            ti += n
    tt = ti

    # instruction plan: per (wi, k) block, runs of <= NI_MAX idxs
    instr_plan = {}          # (wi, k) -> [(tile_off_in_block, ni, idx_col0)]
    idx_cols = 0
    for wi in range(N_WAVES):
        for k in range(N_CHUNK):
            slots = wk_tiles[(wi, k)] * P
            lst = []
            done = 0
            while done < slots:
                ni = min(NI_MAX, slots - done)
                lst.append((done // P, ni, idx_cols))
                idx_cols += ni // 16
                done += ni
            instr_plan[(wi, k)] = lst

    # mw-tile offsets per ordering
    # wave-major (L1): mw tile of wave wi = blocks k=0..3 concatenated
    wm_off = {}
    wm_tiles = []
    for wi in range(N_WAVES):
        o = 0
        for k in range(N_CHUNK):
            wm_off[(wi, k)] = o
            o += wk_tiles[(wi, k)]
        wm_tiles.append(o)
    # chunk-pair-major (L2/L3): mw tile of (pass, wave) = blocks k in pass
    cm_off = {}
    cm_tiles = {}
    for pi, ks in enumerate(PASSES):
        for wi in range(N_WAVES):
            o = 0
            for k in ks:
                cm_off[(pi, wi, k)] = o
                o += wk_tiles[(wi, k)]
            cm_tiles[(pi, wi)] = o
    mw_max = max(max(wm_tiles), max(cm_tiles.values()))

    # per-core idx (int16, wrapped) and doff (fp16) arrays
    idx_arrs, doff_arrs = [], []
    for c in range(N_CORES):
        runs = edge_lists[c]
        flat_idx = np.zeros(tt * P, dtype=np.int16)
        flat_off = np.full(tt * P, -1.0, dtype=np.float32)
        pos = 0
        for (wi, k, g, j) in tiles_meta:
            rows, offs = runs[(g, k)]
            a, b = j * P, min((j + 1) * P, len(rows))
            n = max(0, b - a)
            if n > 0:
                flat_idx[pos:pos + n] = rows[a:b].astype(np.int16)
                flat_off[pos:pos + n] = offs[a:b].astype(np.float16)
            pos += P
        idx_wrapped = np.zeros((P, idx_cols), dtype=np.int16)
        for wi in range(N_WAVES):
            for k in range(N_CHUNK):
                for (toff, ni, col0) in instr_plan[(wi, k)]:
                    s0 = (wk_tilebase[(wi, k)] + toff) * P
                    blk = flat_idx[s0:s0 + ni].reshape(ni // 16, 16).T
                    idx_wrapped[:, col0:col0 + ni // 16] = np.tile(blk, (8, 1))
        idx_arrs.append(idx_wrapped)
        doff_arrs.append(flat_off.reshape(tt, P).T.copy())

    return dict(
        dinv=dinv, perms=perms, t_arr=t_arr, instr_plan=instr_plan,
        tiles_meta=tiles_meta, tt=tt, wk_tilebase=wk_tilebase,
        wk_tiles=wk_tiles, wm_off=wm_off, wm_tiles=wm_tiles,
        cm_off=cm_off, cm_tiles=cm_tiles, mw_max=mw_max,
        idx_arrs=idx_arrs, doff_arrs=doff_arrs, idx_cols=idx_cols,
    )


def _build_program(S, repeat=1, ag_mode='collective'):
    t_arr, instr_plan, tt, idx_cols = (
        S['t_arr'], S['instr_plan'], S['tt'], S['idx_cols'])
    wk_tilebase, wk_tiles = S['wk_tilebase'], S['wk_tiles']
    wm_off, wm_tiles = S['wm_off'], S['wm_tiles']
    cm_off, cm_tiles, mw_max = S['cm_off'], S['cm_tiles'], S['mw_max']

    nc = bacc.Bacc("TRN2", target_bir_lowering=False, debug=False,
                   num_devices=N_CORES)

    xtab_d = nc.dram_tensor("xtab", [V, P], f16, kind="ExternalInput")
    xTs_d = nc.dram_tensor("xTs", [P, NS], f16, kind="ExternalInput")
    dinvrep_d = nc.dram_tensor("dinvrep", [HID_DIM, NS], f16,
                               kind="ExternalInput")
    idx_d = nc.dram_tensor("idx", [P, idx_cols], i16, kind="ExternalInput")
    doff_d = nc.dram_tensor("doff", [P, tt], f32, kind="ExternalInput")
    W1_d = nc.dram_tensor("W1", [IN_DIM, HID_DIM], f16, kind="ExternalInput")
    W2_d = nc.dram_tensor("W2", [HID_DIM, HID_DIM], f16, kind="ExternalInput")
    W3_d = nc.dram_tensor("W3", [HID_DIM, 8], f16, kind="ExternalInput")
    b1_d = nc.dram_tensor("b1", [HID_DIM, 1], f32, kind="ExternalInput")
    b2_d = nc.dram_tensor("b2", [HID_DIM, 1], f32, kind="ExternalInput")
    b3_d = nc.dram_tensor("b3", [8, 1], f32, kind="ExternalInput")
    out_d = nc.dram_tensor("out_shard", [NS, 8], f32, kind="ExternalOutput")

    nc.gpsimd.load_library(library_config.mlp)

    with tile.TileContext(nc) as tc:
        stack = ExitStack()
        # packed shard outputs (node-major) for the collectives.
        # One tile per sub so a sub's AllGather read never blocks the next
        # sub's stores (tile-granular dependency tracking).
        zsh2 = [tc.tile([SUB_ROWS[s], HID_DIM], f16, space="DRAM",
                        name=f"zsh2_{s}")[0] for s in range(N_CHUNK)]
        zsh3 = [tc.tile([SUB_ROWS[s], 8], f16, space="DRAM",
                        name=f"zsh3_{s}")[0] for s in range(N_CHUNK)]
        # feature-major self-term stashes
        zTd2 = tc.tile([HID_DIM, NS], f16, space="DRAM", name="zTd2")[0]
        zTd3 = tc.tile([8, NS], f16, space="DRAM", name="zTd3")[0]
        dramp = stack.enter_context(
            tc.tile_pool(name="dramp", bufs=1, space="DRAM"))

        def alloc_tables(rep):
            """Per repeat: packed AllGather landing buffers + padded tables."""
            packs = [[dramp.tile([CHUNK_ROWS[k], w], f16,
                                 addr_space="Shared",
                                 name=f"pk{rep}_{li}_{k}",
                                 tag=f"pk{li}_{k}")
                      for k in range(N_CHUNK)]
                     for li, w in ((2, HID_DIM), (3, 8))]
            tabs = [[dramp.tile([CHUNK_ROWS[k], P], f16,
                                name=f"tab{rep}_{li}_{k}",
                                tag=f"tab{li}_{k}")
                     for k in range(N_CHUNK)]
                    for li in (2, 3)]
            return packs, tabs

        const = stack.enter_context(tc.tile_pool(name="const", bufs=1))

        R_i = const.tile([P, G], mybir.dt.int32)
        nc.gpsimd.iota(R_i[:], pattern=[[1, G]], base=0, channel_multiplier=0)
        R_f = const.tile([P, G], f16)
        nc.vector.tensor_copy(out=R_f[:], in_=R_i[:])
        ident = const.tile([P, P], f32)
        make_identity(nc, ident[:])
        ident_h = const.tile([P, P], f16)
        nc.vector.tensor_copy(out=ident_h[:], in_=ident[:])
        ident_r = const.tile([P, P], f32r)
        nc.vector.tensor_copy(out=ident_r[:], in_=ident[:])

        W1_t = const.tile([IN_DIM, HID_DIM], f16)
        nc.sync.dma_start(out=W1_t[:], in_=W1_d[:])
        W2_t = const.tile([HID_DIM, HID_DIM], f16)
        nc.sync.dma_start(out=W2_t[:], in_=W2_d[:])
        W3_t = const.tile([HID_DIM, 8], f16)
        nc.sync.dma_start(out=W3_t[:], in_=W3_d[:])
        b1_t = const.tile([HID_DIM, 1], f32)
        nc.sync.dma_start(out=b1_t[:], in_=b1_d[:])
        b2_t = const.tile([HID_DIM, 1], f32)
        nc.sync.dma_start(out=b2_t[:], in_=b2_d[:])
        b3_t = const.tile([8, 1], f32)
        nc.sync.dma_start(out=b3_t[:], in_=b3_d[:])
        idx_t = const.tile([P, idx_cols], i16)
        nc.sync.dma_start(out=idx_t[:], in_=idx_d[:])
        doff_t = const.tile([P, tt], f32)
        nc.sync.dma_start(out=doff_t[:], in_=doff_d[:])
        stg2s = [const.tile([P, (26624 // 4 // P) * P], f16,
                            name=f"stg2_{i}")
                 for i in range(2)]
        for _t in stg2s:
            nc.vector.memset(_t[:], 0.0)

        sbuf = stack.enter_context(tc.tile_pool(name="sbuf", bufs=3))
        spool = stack.enter_context(tc.tile_pool(name="spool", bufs=6))
        wavep = stack.enter_context(tc.tile_pool(name="wavep", bufs=2))
        accp = stack.enter_context(tc.tile_pool(name="accp", bufs=1))
        znodep = stack.enter_context(tc.tile_pool(name="znodep", bufs=2))
        stgp = stack.enter_context(tc.tile_pool(name="stgp", bufs=2))
        psum_agg = stack.enter_context(
            tc.tile_pool(name="psum_agg", bufs=2, space="PSUM"))
        psum_mm2 = stack.enter_context(
            tc.tile_pool(name="psum_mm2", bufs=2, space="PSUM"))
        psum_tr = stack.enter_context(
            tc.tile_pool(name="psum_tr", bufs=2, space="PSUM"))

        def load_dvw(wg):
            w0, wn = wg[0] * G, len(wg) * G
            dvw = wavep.tile([HID_DIM, wn], f16, tag="dvw",
                             padded_shape=[HID_DIM, 5 * G])
            nc.sync.dma_start(out=dvw[:], in_=dinvrep_d[:, w0:w0 + wn])
            return dvw

        def build_S(nS, ft):
            """one-hot S[e, j] = (doff[e] == j), fp16.

            DVE only: gpsimd tensor ops are ~2x slower per element on HW,
            serialize in-order with dma_gather on the Pool queue, and fight
            DVE for the shared SBUF port pair."""
            St = spool.tile([P, G], f16, tag="St")
            nc.vector.tensor_scalar(
                out=St[:], in0=R_f[:],
                scalar1=doff_t[:, ft:ft + 1], scalar2=None,
                op0=mybir.AluOpType.is_equal)
            return St

        def store_wave_fm(zcw, wg, fdim, node_dram, zT_target, dt, idm,
                          row_base=0):
            """Batch-transpose the feature-major wave tile [fdim, wn] into
            node-major [wn, fdim] rows of node_dram (rows offset by
            -row_base); also stash feature-major into zT_target if given."""
            w0, wn = wg[0] * G, len(wg) * G
            sfx = "h" if dt is f16 else "f"
            nch = wn // P
            for blk0 in range(0, nch, 8):
                nb = min(8, nch - blk0)
                ptr = psum_tr.tile([P, nb * fdim], dt, tag="ptr" + sfx,
                                   padded_shape=[P, 8 * HID_DIM])
                for i in range(nb):
                    nc.tensor.transpose(
                        out=ptr[:, i * fdim:(i + 1) * fdim],
                        in_=zcw[:fdim, (blk0 + i) * P:(blk0 + i + 1) * P],
                        identity=idm[:fdim, :fdim])
                zn = znodep.tile([P, nb * fdim], dt, tag="zn" + sfx,
                                 padded_shape=[P, 8 * HID_DIM])
                nc.vector.tensor_copy(out=zn[:], in_=ptr[:])
                r0 = w0 - row_base + blk0 * P
                dst = node_dram[r0: r0 + nb * P, :]
                src_ap = zn[:] if dt is f16 else zn[:].bitcast(f32)
                nc.scalar.dma_start(
                    out=dst.rearrange("(c p) f -> p c f", p=P),
                    in_=src_ap.rearrange("p (c f) -> p c f", f=fdim))
            if zT_target is not None:
                nc.scalar.dma_start(out=zT_target[:, w0:w0 + wn], in_=zcw[:])

        def sub_allgather(zsh_t, pack_k, s):
            rn = SUB_ROWS[s]
            if ag_mode == 'local':
                nc.scalar.dma_start(out=pack_k[0:rn, :], in_=zsh_t[:])
                return
            nc.gpsimd.collective_compute(
                "AllGather", mybir.AluOpType.bypass,
                replica_groups=[list(range(N_CORES))],
                ins=[zsh_t[:]],
                outs=[pack_k[:]])

        def repack(pack_k, tab_k, k, fdim):
            """packed [rows, fdim] f16 -> padded [rows, 128] f16 (256B rows),
            via SBUF: contiguous load, DVE re-stride, contiguous store.
            stg2s are persistent ping-pong tiles (memset once; only the real
            feature columns are rewritten — pad columns carry stale zeros)."""
            rows = CHUNK_ROWS[k]
            nq = 4
            qr = rows // nq
            rpp = qr // P                      # rows per partition
            for q in range(nq):
                stg = stgp.tile([P, rpp * fdim], f16, tag="stg",
                                padded_shape=[P, (26624 // 4 // P) * HID_DIM])
                nc.sync.dma_start(
                    out=stg[:],
                    in_=pack_k[q * qr:(q + 1) * qr, :].rearrange(
                        "(p c) f -> p (c f)", p=P))
                stg2 = stg2s[q % 2]
                nc.vector.tensor_copy(
                    out=stg2[:, 0:rpp * P].rearrange("p (c f) -> p c f", f=P
                                                     )[:, :, 0:fdim],
                    in_=stg[:].rearrange("p (c f) -> p c f", f=fdim))
                nc.scalar.dma_start(
                    out=tab_k[q * qr:(q + 1) * qr, :].rearrange(
                        "(p c) f -> p (c f)", p=P),
                    in_=stg2[:, 0:rpp * P])

        def gathers(mw, wi, ks, in_aps, offs):
            for k, in_ap, off in zip(ks, in_aps, offs):
                for (toff, ni, col0) in instr_plan[(wi, k)]:
                    nc.gpsimd.dma_gather(
                        out_ap=mw[:, off + toff: off + toff + ni // P, :],
                        in_ap=in_ap,
                        idxs_ap=idx_t[:, col0: col0 + ni // 16],
                        num_idxs=ni, num_idxs_reg=ni, elem_size=P,
                        single_packet=True,
                    )

        def agg_mms(ps, fdim, mw, wi, wg, g, ks, offs):
            """accumulate all tiles of group g (chunks ks) into psum ps."""
            n_mm = int(t_arr[g, ks].sum())
            mm_i = 0
            for k, off in zip(ks, offs):
                jbase = int(t_arr[wg[0]:g, k].sum())
                for j in range(int(t_arr[g, k])):
                    wt = off + jbase + j
                    ft = wk_tilebase[(wi, k)] + jbase + j
                    St = build_S(mm_i % 3, ft)
                    nc.tensor.matmul(
                        out=ps[:], lhsT=mw[:, wt, 0:fdim], rhs=St[:],
                        start=(mm_i == 0), stop=(mm_i == n_mm - 1))
                    mm_i += 1

        for _rep in range(repeat):
            packs, tabs = alloc_tables(_rep)
            t2pack, t3pack = packs
            table2, table3 = tabs

            # ---------- L1: wave-major, gather x rows, fold W1/W2 after ----
            for wi, (s, wg) in enumerate(WAVES):
                w0, wn = wg[0] * G, len(wg) * G
                mw = wavep.tile([P, wm_tiles[wi], P], f16, tag="mw",
                                padded_shape=[P, mw_max, P])
                gathers(mw, wi, range(N_CHUNK),
                        [xtab_d[CHUNK_BASE[k]:CHUNK_BASE[k + 1], :]
                         for k in range(N_CHUNK)],
                        [wm_off[(wi, k)] for k in range(N_CHUNK)])
                xsw = wavep.tile([P, wn], f16, tag="xsw",
                                 padded_shape=[P, 5 * G])
                nc.sync.dma_start(out=xsw[:], in_=xTs_d[:, w0:w0 + wn])
                dvw = load_dvw(wg)
                zcw = wavep.tile([HID_DIM, wn], f16, tag="zcw",
                                 padded_shape=[HID_DIM, 5 * G])
                for g in wg:
                    c0 = (g - wg[0]) * G
                    c1 = c0 + G
                    psA = psum_agg.tile([P, G], f32, tag="ps")
                    agg_mms(psA, P, mw, wi, wg, g, list(range(N_CHUNK)),
                            [wm_off[(wi, k)] for k in range(N_CHUNK)])
                    t0 = sbuf.tile([P, G], f16, tag="t0")
                    nc.vector.tensor_tensor(out=t0[:], in0=psA[:],
                                            in1=xsw[:, c0:c1],
                                            op=mybir.AluOpType.add)
                    psB = psum_mm2.tile([HID_DIM, G], f32, tag="pm")
                    nc.tensor.matmul(out=psB[:], lhsT=W1_t[:], rhs=t0[:],
                                     start=True, stop=True)
                    u = sbuf.tile([HID_DIM, G], f16, tag="u")
                    nc.vector.tensor_tensor(out=u[:], in0=psB[:],
                                            in1=dvw[:, c0:c1],
                                            op=mybir.AluOpType.mult)
                    hT = sbuf.tile([HID_DIM, G], f16, tag="hT")
                    nc.vector.tensor_scalar(
                        out=hT[:], in0=u[:], scalar1=b1_t[:, :1],
                        scalar2=0.0, op0=mybir.AluOpType.add,
                        op1=mybir.AluOpType.max)
                    psC = psum_mm2.tile([HID_DIM, G], f32, tag="pm")
                    nc.tensor.matmul(out=psC[:], lhsT=W2_t[:], rhs=hT[:],
                                     start=True, stop=True)
                    nc.vector.tensor_tensor(out=zcw[:, c0:c1], in0=psC[:],
                                            in1=dvw[:, c0:c1],
                                            op=mybir.AluOpType.mult)
                store_wave_fm(zcw, wg, HID_DIM, zsh2[s], zTd2, f16,
                              ident_h, row_base=SUB_R0[s])
                if wi + 1 == N_WAVES or WAVES[wi + 1][0] != s:
                    sub_allgather(zsh2[s], t2pack[s], s)
                    repack(t2pack[s], table2[s], s, HID_DIM)

            # ---------- L2 / L3: chunk-pair passes ----------
            def agg_layer(table, fdim_in, zT_self, final, W_next, bias_t,
                          zsh_out, zT_out, pack_out, tab_out):
                fd_out = 8 if final or W_next is W3_t else HID_DIM
                acc = accp.tile([fdim_in, NS], f16, tag="acc",
                                padded_shape=[HID_DIM, NS])
                for pi, ks in enumerate(PASSES):
                    for wi, (s, wg) in enumerate(WAVES):
                        w0, wn = wg[0] * G, len(wg) * G
                        offs = [cm_off[(pi, wi, k)] for k in ks]
                        mw = wavep.tile([P, cm_tiles[(pi, wi)], P], f16,
                                        tag="mw", padded_shape=[P, mw_max, P])
                        gathers(mw, wi, ks, [table[k][:] for k in ks], offs)
                        if pi == 0:
                            zsw = wavep.tile([fdim_in, wn], f16, tag="zsw",
                                             padded_shape=[HID_DIM, 5 * G])
                            nc.sync.dma_start(out=zsw[:],
                                              in_=zT_self[:, w0:w0 + wn])
                        else:
                            dvw = load_dvw(wg)
                            zcw = wavep.tile([fd_out, wn],
                                             f32r if final else f16,
                                             tag="ocw" if final else "zcw",
                                             padded_shape=[HID_DIM, 5 * G])
                        for g in wg:
                            c0 = (g - wg[0]) * G
                            c1 = c0 + G
                            ps = psum_agg.tile([fdim_in, G], f32, tag="ps",
                                               padded_shape=[P, G])
                            agg_mms(ps, fdim_in, mw, wi, wg, g, list(ks),
                                    offs)
                            ac = acc[:, g * G:(g + 1) * G]
                            if pi == 0:
                                # fold the self term into the accumulator
                                nc.vector.tensor_tensor(
                                    out=ac, in0=ps[:], in1=zsw[:, c0:c1],
                                    op=mybir.AluOpType.add)
                                continue
                            e1 = sbuf.tile([fdim_in, G], f16, tag="e1",
                                           padded_shape=[HID_DIM, G])
                            nc.vector.tensor_tensor(
                                out=e1[:], in0=ps[:], in1=ac,
                                op=mybir.AluOpType.add)
                            if final:
                                u = sbuf.tile([8, G], f32, tag="uf")
                                nc.vector.tensor_tensor(
                                    out=u[:], in0=e1[:], in1=dvw[0:8, c0:c1],
                                    op=mybir.AluOpType.mult)
                                nc.vector.tensor_scalar(
                                    out=zcw[:, c0:c1], in0=u[:],
                                    scalar1=b3_t[:, :1], scalar2=None,
                                    op0=mybir.AluOpType.add)
                            else:
                                u = sbuf.tile([HID_DIM, G], f16, tag="u")
                                nc.vector.tensor_tensor(
                                    out=u[:], in0=e1[:], in1=dvw[:, c0:c1],
                                    op=mybir.AluOpType.mult)
                                hT = sbuf.tile([HID_DIM, G], f16, tag="hT")
                                nc.vector.tensor_scalar(
                                    out=hT[:], in0=u[:], scalar1=bias_t[:, :1],
                                    scalar2=0.0, op0=mybir.AluOpType.add,
                                    op1=mybir.AluOpType.max)
                                psC = psum_mm2.tile([fd_out, G], f32,
                                                    tag="pm",
                                                    padded_shape=[HID_DIM, G])
                                nc.tensor.matmul(out=psC[:], lhsT=W_next[:],
                                                 rhs=hT[:], start=True,
                                                 stop=True)
                                nc.vector.tensor_tensor(
                                    out=zcw[:, c0:c1], in0=psC[:],
                                    in1=dvw[0:fd_out, c0:c1],
                                    op=mybir.AluOpType.mult)
                        if pi == 1:
                            if final:
                                store_wave_fm(zcw, wg, 8, out_d, None,
                                              f32r, ident_r)
                            else:
                                store_wave_fm(zcw, wg, fd_out, zsh_out[s],
                                              zT_out, f16, ident_h,
                                              row_base=SUB_R0[s])
                                if wi + 1 == N_WAVES or WAVES[wi + 1][0] != s:
                                    sub_allgather(zsh_out[s], pack_out[s], s)
                                    repack(pack_out[s], tab_out[s], s, fd_out)

            agg_layer(table2, HID_DIM, zTd2, False, W3_t, b2_t,
                      zsh3, zTd3, t3pack, table3)
            agg_layer(table3, 8, zTd3, True, None, b3_t,
                      None, None, None, None)
        stack.close()

    nc.finalize()
    return nc


def _make_in_maps(S, x, W1, b1, W2, b2, W3, b3):
    dinv = S['dinv']
    W3p = np.zeros((HID_DIM, 8), np.float32)
    W3p[:, :OUT_DIM] = W3
    b3p = np.zeros((8, 1), np.float32)
    b3p[:OUT_DIM, 0] = b3

    # x table: dinv-prescaled x in global table (chunk) order, fp16.
    xs_scaled = x * dinv[:, None]
    xtab = np.zeros((V, P), np.float16)
    dvfull = np.zeros(V, np.float32)
    for c in range(N_CORES):
        perm = S['perms'][c]
        loc = np.zeros((NS, IN_DIM), np.float32)
        dv = np.ones(NS, np.float32)
        loc[perm[:NS_RAW]] = xs_scaled[c * NS_RAW:(c + 1) * NS_RAW]
        dv[perm[:NS_RAW]] = dinv[c * NS_RAW:(c + 1) * NS_RAW]
        # scatter this core's shard rows into the chunked table layout
        for s in range(N_CHUNK):
            r0, rn = SUB_R0[s], SUB_ROWS[s]
            base = CHUNK_BASE[s] + c * rn
            xtab[base:base + rn] = loc[r0:r0 + rn].astype(np.float16)
            dvfull[base:base + rn] = dv[r0:r0 + rn]

    in_maps = []
    for c in range(N_CORES):
        perm = S['perms'][c]
        loc = np.zeros((NS, IN_DIM), np.float32)
        dv = np.ones(NS, np.float32)
        loc[perm[:NS_RAW]] = xs_scaled[c * NS_RAW:(c + 1) * NS_RAW]
        dv[perm[:NS_RAW]] = dinv[c * NS_RAW:(c + 1) * NS_RAW]
        in_maps.append({
            "xtab": xtab,
            "xTs": np.ascontiguousarray(loc.T.astype(np.float16)),
            "dinvrep": np.ascontiguousarray(
                np.broadcast_to(dv[None, :], (HID_DIM, NS))).astype(
                    np.float16),
            "idx": S['idx_arrs'][c],
            "doff": S['doff_arrs'][c],
            "W1": W1.astype(np.float16), "W2": W2.astype(np.float16),
            "W3": W3p.astype(np.float16),
            "b1": b1.reshape(-1, 1),
            "b2": b2.reshape(-1, 1),
            "b3": b3p,
        })
    return in_maps


_LAST = {}


def kernel(x, edge_index, W1, b1, W2, b2, W3, b3):
    x = np.asarray(x, dtype=np.float32)
    W1 = np.asarray(W1, dtype=np.float32)
    W2 = np.asarray(W2, dtype=np.float32)
    W3 = np.asarray(W3, dtype=np.float32)
    b1 = np.asarray(b1, dtype=np.float32)
    b2 = np.asarray(b2, dtype=np.float32)
    b3 = np.asarray(b3, dtype=np.float32)

    S = _host_prep(edge_index)
    nc = _build_program(S)
    in_maps = _make_in_maps(S, x, W1, b1, W2, b2, W3, b3)

    res = run_bass_kernel_spmd(nc, in_maps, core_ids=list(range(N_CORES)))

    _LAST['S'] = S
    _LAST['in_maps'] = in_maps

    out = np.empty((N_NODES, OUT_DIM), np.float32)
    for c in range(N_CORES):
        shard = res.results[c]["out_shard"]       # [NS, 8]
        perm = S['perms'][c]
        out[c * NS_RAW:(c + 1) * NS_RAW] = shard[perm[:NS_RAW], :OUT_DIM]
    return out


def measure_exec_ns(repeats=(1, 5), iters=6, ag_mode='collective'):
    """Estimate HW exec time by building R-times-repeated variants of the
    full pipeline and differencing pipelined wall-clock."""
    import time
    import jax
    from jax.sharding import Mesh, PartitionSpec, NamedSharding
    from jax.experimental.shard_map import shard_map
    from concourse import bass2jax
    from concourse.bass2jax import _bass_exec_p, install_neuronx_cc_hook

    S, in_maps = _LAST['S'], _LAST['in_maps']
    install_neuronx_cc_hook()
    per_call = {}
    for R in repeats:
        nc = _build_program(S, repeat=R, ag_mode=ag_mode)
        partition_name = (nc.partition_id_tensor.name
                          if nc.partition_id_tensor else None)
        in_names, out_names, out_avals, zero_outs = [], [], [], []
        for alloc in nc.m.functions[0].allocations:
            if not isinstance(alloc, mybir.MemoryLocationSet):
                continue
            name = alloc.memorylocations[0].name
            if alloc.kind == "ExternalInput":
                if name != partition_name:
                    in_names.append(name)
            elif alloc.kind == "ExternalOutput":
                out_names.append(name)
                shape = tuple(alloc.tensor_shape)
                dtype = mybir.dt.np(alloc.dtype)
                out_avals.append(jax.core.ShapedArray(shape, dtype))
                zero_outs.append(np.zeros(shape, dtype))
        all_in = list(in_names) + list(out_names)
        if partition_name:
            all_in.append(partition_name)

        def _body(*args, _nc=nc, _avals=tuple(out_avals), _in=tuple(all_in),
                  _out=tuple(out_names)):
            operands = list(args)
            operands.append(bass2jax.partition_id_tensor())
            return tuple(_bass_exec_p.bind(
                *operands, out_avals=_avals, in_names=_in, out_names=_out,
                lowering_input_output_aliases=(), sim_require_finite=True,
                sim_require_nnan=True, nc=_nc))

        devices = jax.devices()[:N_CORES]
        mesh = Mesh(np.asarray(devices), ("core",))
        nsp = len(in_names) + len(zero_outs)
        sharded = jax.jit(shard_map(
            _body, mesh=mesh, in_specs=(PartitionSpec("core"),) * nsp,
            out_specs=(PartitionSpec("core"),) * len(out_names),
            check_rep=False), keep_unused=True)
        args = [np.concatenate([np.asarray(in_maps[c][n]) for c in
                                range(N_CORES)], axis=0) for n in in_names]
        args += [np.zeros((N_CORES * z.shape[0], *z.shape[1:]), z.dtype)
                 for z in zero_outs]
        sh = NamedSharding(mesh, PartitionSpec("core"))
        args = [jax.device_put(a, sh) for a in args]
        outs = sharded(*args)
        jax.block_until_ready(outs)
        best = None
        for _ in range(iters):
            t0 = time.perf_counter()
            got = [sharded(*args) for _ in range(4)]
            jax.block_until_ready(got)
            dt = (time.perf_counter() - t0) / 4
            best = dt if best is None else min(best, dt)
        per_call[R] = best
    r0, r1 = repeats
    est = (per_call[r1] - per_call[r0]) / (r1 - r0)
    return max(1, int(est * 1e9))
